# revision 8
# baseline (speedup 1.0000x reference)
"""Trainium2 Bass kernel for AffineMultiQueryHardAttentionEncoder.

reference:
    scores = max_n( (queries * affine) @ keys.T )        # [M]
    w, idx = top_k(scores, 64); w = softmax(w)
    encoding = sum(w[:, None] * values[idx], 0)          # [D]
    returns (encoding, idx)

Strategy (8 NeuronCores):
  Launch 1 (SPMD x8): shard keys along M (12500/core, padded to 12800).
    Each core: bf16 matmul scan  scores[m] = max_n (q*a). k_m  with m on
    partitions / n on the free dim, DVE reduce_max over n, then per-partition
    top-16 candidates via the DVE max8/match_replace/max_index primitives.
    -> 2048 candidate (value, index) pairs per core.
  Host: merge 8*2048 candidates, keep global top-128 by (bf16-noisy) score,
    gather those keys/values rows (data movement only).
  Launch 2 (1 core): recompute the 128 candidate scores exactly in fp32,
    exact top-64 + index recovery + softmax + weighted sum of value rows,
    all on device.

The bf16 scan only needs to rank well enough that the true top-64 are inside
the noisy top-128 of the union of per-partition top-16s; margins here are
enormous (bf16 score noise ~0.15 abs vs. candidate-rank spreads of many
units; fp32 recompute restores exact ordering/weights).
"""

import os
import numpy as np
import ml_dtypes

import concourse.bass as bass
import concourse.bacc as bacc
import concourse.mybir as mybir
from concourse.tile import TileContext
from concourse.bass_utils import run_bass_kernel_spmd

BF16 = ml_dtypes.bfloat16
F32 = mybir.dt.float32
U32 = mybir.dt.uint32

N_CORES = 8
N = 512          # queries
M = 100000       # keys/values rows
D = 1024         # feature dim
KTOP = 64
DCH = D // 128   # 8 contraction chunks

M_SHARD = M // N_CORES          # 12500
M_PAD = 12800                   # per-core padded key count
SWATH = 1280                    # m columns per DMA swath
NSW = M_PAD // SWATH            # 10
MT_PER_SW = SWATH // 128        # 10
NT = M_PAD // 128               # 100 m-tiles -> scores [128, NT]

CAND_ROUNDS = 2                 # per-partition top-(8*rounds)
CAND_PER_P = 8 * CAND_ROUNDS    # 16
MARGIN = 128                    # global noisy-candidate count re-scored exactly

NEG = -1.0e30

_cache: dict = {}


def _build_scan(m_pad=M_PAD, swath=SWATH):
    """Launch-1 program: bf16 score scan + per-partition top-16 candidates."""
    nsw = m_pad // swath
    mt_per_sw = swath // 128
    nt = m_pad // 128
    # padded tail inside the scores tile: m_local = t*128 + p >= M_SHARD
    pad_t0, pad_p0 = divmod(M_SHARD, 128) if m_pad > M_SHARD else (nt, 0)

    nc = bacc.Bacc("TRN2", debug=False)
    kT = nc.dram_tensor("keysT", [D, m_pad], mybir.dt.bfloat16, kind="ExternalInput")
    qT = nc.dram_tensor("queriesT", [D, N], F32, kind="ExternalInput")
    aff = nc.dram_tensor("affpc", [128, DCH], F32, kind="ExternalInput")
    o_vals = nc.dram_tensor("cand_vals", [128, CAND_PER_P], F32, kind="ExternalOutput")
    o_idx = nc.dram_tensor("cand_idx", [128, CAND_PER_P], U32, kind="ExternalOutput")

    with TileContext(nc) as tc:
        with (
            tc.tile_pool(name="const", bufs=1) as cpool,
            tc.tile_pool(name="keys", bufs=2) as kpool,
            tc.tile_pool(name="ps", bufs=4, space="PSUM") as ppool,
            tc.tile_pool(name="wk", bufs=1) as wpool,
        ):
            afft = cpool.tile([128, DCH], F32)
            nc.sync.dma_start(out=afft, in_=aff.ap())
            qf = cpool.tile([128, DCH * N], F32)
            for d in range(DCH):
                nc.sync.dma_start(
                    out=qf[:, d * N:(d + 1) * N],
                    in_=qT.ap()[d * 128:(d + 1) * 128, :],
                )
            # q'_bf16[d] = bf16(affine[d-chunk] * qT[d-chunk])
            qb = cpool.tile([128, DCH * N], mybir.dt.bfloat16)
            for d in range(DCH):
                nc.vector.tensor_scalar_mul(
                    qb[:, d * N:(d + 1) * N],
                    qf[:, d * N:(d + 1) * N],
                    afft[:, d:d + 1],
                )

            scores = wpool.tile([128, nt], F32)
            for s in range(nsw):
                kt = [
                    kpool.tile([128, swath], mybir.dt.bfloat16,
                               name=f"kt{d}", tag=f"k{d}")
                    for d in range(DCH)
                ]
                for d in range(DCH):
                    nc.sync.dma_start(
                        out=kt[d],
                        in_=kT.ap()[d * 128:(d + 1) * 128, s * swath:(s + 1) * swath],
                    )
                for mt in range(mt_per_sw):
                    ps = ppool.tile([128, N], F32)
                    for d in range(DCH):
                        nc.tensor.matmul(
                            ps[:],
                            lhsT=kt[d][:, mt * 128:(mt + 1) * 128],
                            rhs=qb[:, d * N:(d + 1) * N],
                            start=(d == 0),
                            stop=(d == DCH - 1),
                        )
                    t = s * mt_per_sw + mt
                    nc.vector.reduce_max(
                        out=scores[:, t:t + 1], in_=ps[:], axis=mybir.AxisListType.X
                    )

            # Padded key columns are zero -> score exactly 0, while every real
            # score is the max of 512 ~N(0, 32) dots (>= +60 in practice), so
            # pads can never enter a per-partition top-16. The host also
            # filters m >= M as a belt-and-braces guard.

            cv = wpool.tile([128, CAND_PER_P], F32)
            ci = wpool.tile([128, CAND_PER_P], U32)
            for r in range(CAND_ROUNDS):
                sl = slice(r * 8, (r + 1) * 8)
                nc.vector.max(out=cv[:, sl], in_=scores[:])
                nc.vector.max_index(out=ci[:, sl], in_max=cv[:, sl], in_values=scores[:])
                if r < CAND_ROUNDS - 1:
                    nc.vector.match_replace(
                        out=scores[:], in_to_replace=cv[:, sl],
                        in_values=scores[:], imm_value=NEG,
                    )
            nc.sync.dma_start(out=o_vals.ap(), in_=cv)
            nc.sync.dma_start(out=o_idx.ap(), in_=ci)
    nc.compile()
    return nc


def _build_combine():
    """Launch-2 program (single core): exact fp32 re-score of MARGIN candidates,
    exact top-64, index recovery, softmax, weighted sum of value rows."""
    nc = bacc.Bacc("TRN2", debug=False)
    kTc = nc.dram_tensor("kTc", [D, MARGIN], F32, kind="ExternalInput")
    vc = nc.dram_tensor("vc", [MARGIN, D], F32, kind="ExternalInput")
    mi = nc.dram_tensor("midx", [MARGIN, 1], F32, kind="ExternalInput")
    qT = nc.dram_tensor("queriesT", [D, N], F32, kind="ExternalInput")
    aff = nc.dram_tensor("affpc", [128, DCH], F32, kind="ExternalInput")
    iden = nc.dram_tensor("iden", [128, 128], F32, kind="ExternalInput")
    o_enc = nc.dram_tensor("enc", [1, D], F32, kind="ExternalOutput")
    o_ind = nc.dram_tensor("ind", [1, KTOP], F32, kind="ExternalOutput")
    o_top = nc.dram_tensor("topvals", [1, KTOP], F32, kind="ExternalOutput")

    with TileContext(nc) as tc:
        with (
            tc.tile_pool(name="c2", bufs=1) as cpool,
            tc.tile_pool(name="p2", bufs=1, space="PSUM") as ppool,
        ):
            afft = cpool.tile([128, DCH], F32)
            nc.sync.dma_start(out=afft, in_=aff.ap())
            qf = cpool.tile([128, DCH * N], F32)
            for d in range(DCH):
                nc.sync.dma_start(
                    out=qf[:, d * N:(d + 1) * N],
                    in_=qT.ap()[d * 128:(d + 1) * 128, :],
                )
            qa = cpool.tile([128, DCH * N], F32)
            for d in range(DCH):
                nc.vector.tensor_scalar_mul(
                    qa[:, d * N:(d + 1) * N],
                    qf[:, d * N:(d + 1) * N],
                    afft[:, d:d + 1],
                )
            ktile = cpool.tile([128, DCH * MARGIN], F32)
            for d in range(DCH):
                nc.sync.dma_start(
                    out=ktile[:, d * MARGIN:(d + 1) * MARGIN],
                    in_=kTc.ap()[d * 128:(d + 1) * 128, :],
                )
            vt = cpool.tile([128, D], F32)
            nc.sync.dma_start(out=vt, in_=vc.ap())
            mit = cpool.tile([128, 1], F32)
            nc.sync.dma_start(out=mit, in_=mi.ap())
            idt = cpool.tile([128, 128], F32)
            nc.sync.dma_start(out=idt, in_=iden.ap())

            # exact fp32 scores of the MARGIN candidates
            ps = ppool.tile([128, N], F32, tag="score")
            for d in range(DCH):
                nc.tensor.matmul(
                    ps[:],
                    lhsT=ktile[:, d * MARGIN:(d + 1) * MARGIN],
                    rhs=qa[:, d * N:(d + 1) * N],
                    start=(d == 0),
                    stop=(d == DCH - 1),
                )
            sc = cpool.tile([128, 1], F32)
            nc.vector.reduce_max(out=sc, in_=ps[:], axis=mybir.AxisListType.X)

            # transpose scores [128,1] -> [1,128] via identity matmul
            pst = ppool.tile([1, 128], F32, tag="tr")
            nc.tensor.matmul(pst[:], lhsT=sc[:], rhs=idt[:], start=True, stop=True)
            scr = cpool.tile([1, 128], F32)
            nc.vector.tensor_copy(out=scr, in_=pst[:])

            # exact, sorted top-64 of the 128 candidate scores
            tv = cpool.tile([1, KTOP], F32)
            for r in range(KTOP // 8):
                sl = slice(r * 8, (r + 1) * 8)
                nc.vector.max(out=tv[:, sl], in_=scr[:])
                if r < KTOP // 8 - 1:
                    nc.vector.match_replace(
                        out=scr[:], in_to_replace=tv[:, sl],
                        in_values=scr[:], imm_value=NEG,
                    )
            nc.sync.dma_start(out=o_top.ap(), in_=tv)

            # eq[p, j] = (score[p] == top64[j]) ; broadcast top64 to all partitions
            ones_r = cpool.tile([1, 128], F32)
            nc.vector.memset(ones_r, 1.0)
            psb = ppool.tile([128, KTOP], F32, tag="bcast")
            nc.tensor.matmul(psb[:], lhsT=ones_r[:], rhs=tv[:], start=True, stop=True)
            t64b = cpool.tile([128, KTOP], F32)
            nc.vector.tensor_copy(out=t64b, in_=psb[:])
            eq = cpool.tile([128, KTOP], F32)
            nc.vector.tensor_tensor(
                eq[:], sc.to_broadcast([128, KTOP]), t64b[:], mybir.AluOpType.is_equal
            )

            # indices: ind[j] = sum_p eq[p, j] * m_idx[p]
            tmpi = cpool.tile([128, KTOP], F32)
            nc.vector.tensor_scalar_mul(tmpi[:], eq[:], mit[:, 0:1])
            ones = cpool.tile([128, 1], F32)
            nc.vector.memset(ones, 1.0)
            psi = ppool.tile([1, KTOP], F32, tag="ind")
            nc.tensor.matmul(psi[:], lhsT=ones[:], rhs=tmpi[:], start=True, stop=True)
            indr = cpool.tile([1, KTOP], F32)
            nc.vector.tensor_copy(out=indr, in_=psi[:])
            nc.sync.dma_start(out=o_ind.ap(), in_=indr)

            # softmax over the sorted top-64 (tv[0,0] is the max)
            nmax = cpool.tile([1, 1], F32)
            nc.vector.tensor_scalar_mul(nmax, tv[:, 0:1], -1.0)
            ex = cpool.tile([1, KTOP], F32)
            nc.scalar.activation(
                out=ex[:], in_=tv[:], func=mybir.ActivationFunctionType.Exp,
                bias=nmax[:, 0:1], scale=1.0,
            )
            ssum = cpool.tile([1, 1], F32)
            nc.vector.reduce_sum(out=ssum, in_=ex[:], axis=mybir.AxisListType.X)
            rs = cpool.tile([1, 1], F32)
            nc.vector.reciprocal(out=rs, in_=ssum[:])
            wt = cpool.tile([1, KTOP], F32)
            nc.vector.tensor_scalar_mul(wt[:], ex[:], rs[:, 0:1])

            # per-candidate weight: w_cand[p] = sum_j eq[p,j] * w[j]
            psw = ppool.tile([128, KTOP], F32, tag="bcast")
            nc.tensor.matmul(psw[:], lhsT=ones_r[:], rhs=wt[:], start=True, stop=True)
            wb = cpool.tile([128, KTOP], F32)
            nc.vector.tensor_copy(out=wb, in_=psw[:])
            wsel = cpool.tile([128, KTOP], F32)
            nc.vector.tensor_tensor(wsel[:], eq[:], wb[:], mybir.AluOpType.mult)
            wc = cpool.tile([128, 1], F32)
            nc.vector.reduce_sum(out=wc, in_=wsel[:], axis=mybir.AxisListType.X)

            # encoding = w_cand^T @ V   (contraction over the 128 candidates)
            enc = cpool.tile([1, D], F32)
            for h in range(2):
                pse = ppool.tile([1, 512], F32, tag="enc")
                nc.tensor.matmul(
                    pse[:], lhsT=wc[:], rhs=vt[:, h * 512:(h + 1) * 512],
                    start=True, stop=True,
                )
                nc.vector.tensor_copy(out=enc[:, h * 512:(h + 1) * 512], in_=pse[:])
            nc.sync.dma_start(out=o_enc.ap(), in_=enc)
    nc.compile()
    return nc


def _get_programs():
    if "scan" not in _cache:
        _cache["scan"] = _build_scan()
    if "combine" not in _cache:
        _cache["combine"] = _build_combine()
    return _cache["scan"], _cache["combine"]


def kernel(queries, keys, values, affine):
    queries = np.asarray(queries, dtype=np.float32)
    keys = np.asarray(keys, dtype=np.float32)
    values = np.asarray(values, dtype=np.float32)
    affine = np.asarray(affine, dtype=np.float32)

    trace = bool(int(os.environ.get("KERNEL_TRACE", "0")))
    nc1, nc2 = _get_programs()

    qT = np.ascontiguousarray(queries.T)                 # [D, N]
    affpc = np.ascontiguousarray(affine.reshape(DCH, 128).T)  # [128, DCH]
    kT_all = keys.T.astype(BF16)                         # [D, M]

    in_maps = []
    for c in range(N_CORES):
        kc = np.zeros((D, M_PAD), dtype=BF16)
        kc[:, :M_SHARD] = kT_all[:, c * M_SHARD:(c + 1) * M_SHARD]
        in_maps.append({"keysT": kc, "queriesT": qT, "affpc": affpc})

    r1 = run_bass_kernel_spmd(nc1, in_maps, list(range(N_CORES)), trace=trace)
    _cache["last_scan_results"] = r1

    vals = np.stack([r1.results[c]["cand_vals"] for c in range(N_CORES)])  # [8,128,16]
    idxt = np.stack([r1.results[c]["cand_idx"] for c in range(N_CORES)])   # [8,128,16]
    p_arr = np.arange(128, dtype=np.int64)[None, :, None]
    c_arr = np.arange(N_CORES, dtype=np.int64)[:, None, None]
    m_glob = idxt.astype(np.int64) * 128 + p_arr + c_arr * M_SHARD
    vals_f = vals.reshape(-1)
    m_f = m_glob.reshape(-1)
    ok = m_f < M                      # paranoia; padding is already masked
    vals_f, m_f = vals_f[ok], m_f[ok]
    sel = np.argsort(-vals_f, kind="stable")[:MARGIN]
    m_sel = m_f[sel]

    kTc = np.ascontiguousarray(keys[m_sel].T)            # [D, MARGIN]
    vcand = np.ascontiguousarray(values[m_sel])          # [MARGIN, D]
    midx = m_sel.astype(np.float32)[:, None]             # [MARGIN, 1]
    iden = np.eye(128, dtype=np.float32)

    in2 = {"kTc": kTc, "vc": vcand, "midx": midx,
           "queriesT": qT, "affpc": affpc, "iden": iden}
    r2 = run_bass_kernel_spmd(nc2, [in2], [0], trace=trace)
    _cache["last_combine_results"] = r2

    encoding = np.asarray(r2.results[0]["enc"][0], dtype=np.float32)
    indices = np.rint(np.asarray(r2.results[0]["ind"][0])).astype(np.int32)
    return encoding, indices
